# revision 3
# baseline (speedup 1.0000x reference)
"""Trainium2 Bass kernel for the span-search problem (nn_DCR_21285857919673).

Data-parallel over batch: 32 batches / 8 cores = 4 per core. Per batch:
  n2[s]      square+accumulate over H (split ACT activation / DVE stt)
  d1,d2      PE: transpose 128x128 blocks -> PSUM -> SBUF, then reversed
             matmuls (lhsT = transposed block, rhs = [q1,q2]) giving
             d in [128(s), 2] layout directly (output free size 2).
  banded window (start i, len l<32) built ON-CHIP with PE shift-matmuls:
             win[p,r,c,l] = dn[p+l, r, c] with column wrap, via two
             identity-slice pieces per shift accumulated in one PSUM bank.
  mask via additive penalty constants; max / first-argmax / validity via
  DVE reductions + Pool elementwise; sqrt on ACT; Newton-refined divide.
The per-batch band phase is software-pipelined one batch behind the heavy
phase so long-latency waits never plug the in-order engine queues.
All sep-dependent values (q vectors, penalties, validity masks) are shipped
as one packed per-core constant tensor (same SPMD program on all 8 cores).
"""
import sys

sys.path.insert(0, "/opt/trn_rl_repo")

import numpy as np

import concourse.bass as bass
import concourse.bacc as bacc
import concourse.mybir as mybir
import concourse.tile as tile
from concourse.alu_op_type import AluOpType
from concourse import bass_isa
from concourse.bass_utils import run_bass_kernel_spmd

F32 = mybir.dt.float32
BF16 = mybir.dt.bfloat16
I32 = mybir.dt.int32

B_PER_CORE = 4
S = 1024
H = 1024
L = 32
NT = S // 128   # token tiles per batch
NC = H // 128   # h chunks
NEG = -10000.0
PENALTY = -16384.0   # additive mask penalty (exact in f32; way below any sim)

# consts column layout
QM = 0                    # qmat: (b*NC+c)*2+q            -> 64
ID = QM + B_PER_CORE * NC * 2        # ident 128
C1O = ID + 128                       # c1 160
C2O = C1O + 160                      # c2 160
PEN = C2O + 160                      # penj: 256*b + 32*c + l -> 1024
RIO = PEN + B_PER_CORE * NT * L      # riota 256
CC = RIO + NT * L                    # cconst 8
VMF = CC + NT                        # vmif 32
NM1 = VMF + B_PER_CORE * NT          # negm1 32
CCV = NM1 + B_PER_CORE * NT          # cconst*vmif + negm2 32
VVR = CCV + B_PER_CORE * NT          # rsq*vmif 32
CW = VVR + B_PER_CORE * NT           # total 1928

_cache = {}


def _bc(ap, n_mid, width):
    """[128, n_mid] AP -> [128, n_mid, width] broadcast along a new last dim."""
    return bass.AP(ap.tensor, ap.offset, [[ap.ap[0][0], 128], [1, n_mid], [0, width]])


def _build():
    nc = bacc.Bacc("TRN2", target_bir_lowering=False, debug=False)

    seq_in = nc.dram_tensor("seq", [B_PER_CORE, S, H], F32, kind="ExternalInput").ap()
    consts_in = nc.dram_tensor("consts", [128, CW], F32, kind="ExternalInput").ap()
    mv_out = nc.dram_tensor("mv_out", [B_PER_CORE, S], F32, kind="ExternalOutput").ap()
    ei_out = nc.dram_tensor("ei_out", [B_PER_CORE, S], I32, kind="ExternalOutput").ap()

    with tile.TileContext(nc) as tc:
        with (
            tc.tile_pool(name="inp", bufs=32) as inp,
            tc.tile_pool(name="cons", bufs=1) as cons,
            tc.tile_pool(name="trash", bufs=3) as trash_p,
            tc.tile_pool(name="stp", bufs=9) as stp,
            tc.tile_pool(name="band", bufs=3) as band_p,
            tc.tile_pool(name="small", bufs=3) as small_p,
            tc.tile_pool(name="outacc", bufs=1) as outacc,
            tc.tile_pool(name="tpp", bufs=4, space="PSUM") as tpp,
            tc.tile_pool(name="dpp", bufs=2, space="PSUM") as dpp,
            tc.tile_pool(name="wpp", bufs=2, space="PSUM") as wpp,
        ):
            cst = cons.tile([128, CW], F32, tag="cst")
            # split consts load: header (q/ident/shift mats) gates compute,
            # the rest (penalties etc) is only needed by the band phase
            nc.sync.dma_start(cst[:, 0:PEN], consts_in[:, 0:PEN])
            ident = cst[:, ID : ID + 128]
            c1 = cst[:, C1O : C1O + 160]
            c2 = cst[:, C2O : C2O + 160]
            riota3 = cst[:, RIO : RIO + NT * L].rearrange("p (c l) -> p c l", l=L)
            cconst = cst[:, CC : CC + NT]

            mvacc = outacc.tile([128, B_PER_CORE, NT], F32, tag="mvacc")
            eiacc = outacc.tile([128, B_PER_CORE, NT], I32, tag="eiacc")

            state = {}

            def emit_loads(b):
                tiles = []
                for t in range(NT):
                    tl = inp.tile([128, H], F32, tag="seq", name=f"tl{b}_{t}")
                    nc.sync.dma_start(tl[:], seq_in[b, bass.ts(t, 128), :])
                    tiles.append(tl[:])
                if b == 0:
                    nc.sync.dma_start(cst[:, PEN:CW], consts_in[:, PEN:CW])
                dn = small_p.tile([128, 2, NT + 1], F32, tag="dn", name=f"dn{b}")
                nc.gpsimd.memset(dn[:, :, NT : NT + 1], 0.0)
                for t in range(NT):
                    tr = trash_p.tile([128, H], BF16, tag="tr")
                    acc = dn[:, 1, t : t + 1]
                    if t % 2 == 0:
                        nc.scalar.activation(tr[:], tiles[t],
                                             mybir.ActivationFunctionType.Square,
                                             accum_out=acc)
                    else:
                        nc.vector.scalar_tensor_tensor(
                            out=tr[:], in0=tiles[t], scalar=1.0,
                            in1=tiles[t], op0=AluOpType.mult,
                            op1=AluOpType.mult, accum_out=acc)
                sts = [stp.tile([128, 1024], F32, tag="st", name=f"st{b}_{c}")
                       for c in range(NC)]
                dps = dpp.tile([128, NT, 2], F32, tag="dps", name=f"dps{b}")
                state[b] = dict(tiles=tiles, dn=dn, sts=sts, dps=dps)

            def emit_half(b, half):
                st_ = state[b]
                tiles, dn, sts, dps = st_["tiles"], st_["dn"], st_["sts"], st_["dps"]
                qmat_b = cst[:, QM + b * NC * 2 : QM + (b + 1) * NC * 2]
                for c in range(NC):
                    tp = tpp.tile([128, 512], F32, tag="tp")
                    for tt in range(4):
                        t = 4 * half + tt
                        nc.tensor.transpose(tp[:, bass.ts(tt, 128)],
                                            tiles[t][:, bass.ts(c, 128)], ident)
                    dst = sts[c][:, half * 512 : half * 512 + 512]
                    if c < 5 or (c == 5 and half == 0):
                        nc.scalar.copy(dst, tp[:])
                    else:
                        nc.vector.tensor_copy(dst, tp[:])
                for t in range(4 * half, 4 * half + 4):
                    for c in range(NC):
                        nc.tensor.matmul(dps[:, t, :],
                                         lhsT=sts[c][:, bass.ts(t, 128)],
                                         rhs=qmat_b[:, 2 * c : 2 * c + 2],
                                         start=(c == 0), stop=(c == NC - 1))
                if half == 1:
                    d1col = small_p.tile([128, NT], F32, tag="d1col",
                                         name=f"d1c{b}")
                    nc.vector.tensor_copy(d1col[:], dps[:, :, 0])
                    nc.vector.tensor_copy(dn[:, 0, 0:NT], dps[:, :, 1])
                    st_["d1col"] = d1col

            def emit_win_n2row(b):
                st_ = state[b]
                dn = st_["dn"]
                win = wpp.tile([128, 2, NT, L], F32, tag="win", name=f"win{b}")
                # n2-row shifts first: they only need the n2 accumulations,
                # so den/rec can start while the d-row still waits on copies
                for l in range(L):
                    nc.tensor.matmul(win[:, 1, :, l], lhsT=c1[:, l : l + 128],
                                     rhs=dn[:, 1:2, 0:NT],
                                     start=(l == 0), stop=False)
                    nc.tensor.matmul(win[:, 1, :, l], lhsT=c2[:, l : l + 128],
                                     rhs=dn[:, 1:2, 1 : NT + 1],
                                     start=False, stop=False)
                n2ibc = bass.AP(dn.tensor, dn[:].offset + NT + 1,
                                [[dn[:].ap[0][0], 128], [1, NT], [0, L]])
                nc.tensor.matmul(win[:, 1], lhsT=ident, rhs=n2ibc,
                                 start=False, stop=False)
                st_["win"] = win

            def emit_win_drow(b):
                st_ = state[b]
                dn, win, d1col = st_["dn"], st_["win"], st_["d1col"]
                penj3 = cst[:, PEN + b * NT * L : PEN + (b + 1) * NT * L] \
                    .rearrange("p (c l) -> p c l", l=L)
                nc.tensor.matmul(win[:, 0], lhsT=ident, rhs=penj3,
                                 start=False, stop=False)
                nc.tensor.matmul(win[:, 0], lhsT=ident, rhs=_bc(d1col[:], NT, L),
                                 start=False, stop=False)
                for l in range(L):
                    nc.tensor.matmul(win[:, 0, :, l], lhsT=c1[:, l : l + 128],
                                     rhs=dn[:, 0:1, 0:NT],
                                     start=False, stop=False)
                    nc.tensor.matmul(win[:, 0, :, l], lhsT=c2[:, l : l + 128],
                                     rhs=dn[:, 0:1, 1 : NT + 1],
                                     start=False, stop=(l == L - 1))

            def emit_band1(b):
                st_ = state[b]
                win = st_["win"]
                den = band_p.tile([128, NT, L], F32, tag="den")
                nc.scalar.sqrt(den[:], win[:, 1])
                rec = band_p.tile([128, NT, L], F32, tag="rec")
                nc.vector.reciprocal(rec[:], den[:])
                st_.update(den=den, rec=rec)

            def emit_band2(b):
                st_ = state[b]
                numer, den, rec, rsq = st_["numer"], st_["den"], st_["rec"], st_["rsq"]
                vmif_b = cst[:, VMF + b * NT : VMF + (b + 1) * NT]
                negm1_b = cst[:, NM1 + b * NT : NM1 + (b + 1) * NT]
                ccvn_b = cst[:, CCV + b * NT : CCV + (b + 1) * NT]
                vvr_b = cst[:, VVR + b * NT : VVR + (b + 1) * NT]
                sim0 = band_p.tile([128, NT, L], F32, tag="sim0")
                nc.gpsimd.tensor_tensor(out=sim0[:], in0=numer[:], in1=rec[:],
                                        op=AluOpType.mult)
                nt1 = band_p.tile([128, NT, L], F32, tag="nt1")
                nc.gpsimd.tensor_tensor(out=nt1[:], in0=sim0[:], in1=den[:],
                                        op=AluOpType.mult)
                nt2 = band_p.tile([128, NT, L], F32, tag="nt2")
                nc.gpsimd.tensor_tensor(out=nt2[:], in0=numer[:], in1=nt1[:],
                                        op=AluOpType.subtract)
                nt3 = band_p.tile([128, NT, L], F32, tag="nt3")
                nc.gpsimd.tensor_tensor(out=nt3[:], in0=nt2[:], in1=rec[:],
                                        op=AluOpType.mult)
                sim = band_p.tile([128, NT, L], F32, tag="sim")
                nc.gpsimd.tensor_tensor(out=sim[:], in0=sim0[:], in1=nt3[:],
                                        op=AluOpType.add)
                maxv = small_p.tile([128, NT], F32, tag="maxv")
                nc.vector.tensor_reduce(out=maxv[:], in_=sim[:],
                                        axis=mybir.AxisListType.X, op=AluOpType.max)
                eq = band_p.tile([128, NT, L], F32, tag="eq")
                nc.vector.tensor_tensor(out=eq[:], in0=sim[:],
                                        in1=_bc(maxv[:], NT, L), op=AluOpType.is_equal)
                wt = band_p.tile([128, NT, L], F32, tag="wt")
                nc.gpsimd.tensor_tensor(out=wt[:], in0=eq[:], in1=riota3,
                                        op=AluOpType.mult)
                mval = small_p.tile([128, NT], F32, tag="mval")
                nc.vector.tensor_reduce(out=mval[:], in_=wt[:],
                                        axis=mybir.AxisListType.X, op=AluOpType.max)
                mvt = small_p.tile([128, NT], F32, tag="mvt")
                eng.tensor_tensor(out=mvt[:], in0=maxv[:], in1=vvr_b,
                                  op=AluOpType.mult)
                eng.tensor_tensor(out=mvacc[:, b, :], in0=mvt[:], in1=negm1_b,
                                  op=AluOpType.add)
                vm1 = small_p.tile([128, NT], F32, tag="vm1")
                eng.tensor_tensor(out=vm1[:], in0=mval[:], in1=vmif_b,
                                  op=AluOpType.mult)
                ef2 = small_p.tile([128, NT], F32, tag="ef2")
                eng.tensor_tensor(out=ef2[:], in0=ccvn_b, in1=vm1[:],
                                  op=AluOpType.subtract)
                nc.gpsimd.tensor_copy(eiacc[:, b, :], ef2[:])
                del state[b]

            # fine-grained software pipeline
            for b in range(B_PER_CORE):
                emit_loads(b)
                if b > 0:
                    emit_band1(b - 1)
                emit_qn2(b)
                emit_half(b, 0)
                if b > 0:
                    emit_band2(b - 1)
                emit_half(b, 1)
                emit_win(b)
            emit_band1(B_PER_CORE - 1)
            emit_band2(B_PER_CORE - 1)

    nc.compile()
    return nc


def _prep_core(seq_c, sep0_c, sep1_c):
    """Host-side constant prep for one core. seq_c: [4, S, H] f32."""
    Bc = seq_c.shape[0]
    p = np.arange(128)
    cst = np.zeros((128, CW), np.float32)
    for b in range(Bc):
        q1 = seq_c[b, 1, :]
        q2 = seq_c[b, int(sep0_c[b]) - 1, :]
        for c in range(NC):
            cst[:, QM + (b * NC + c) * 2] = q1[c * 128 : (c + 1) * 128]
            cst[:, QM + (b * NC + c) * 2 + 1] = q2[c * 128 : (c + 1) * 128]
        i_all = p[:, None, None] + 128 * np.arange(NT)[None, :, None]  # [128, NT, 1]
        j_all = i_all + np.arange(L)[None, None, :]                    # [128, NT, L]
        pen = np.where(j_all < int(sep1_c[b]), 0.0, PENALTY).astype(np.float32)
        cst[:, PEN + b * NT * L : PEN + (b + 1) * NT * L] = pen.reshape(128, NT * L)
        valid_i = ((i_all[:, :, 0] > int(sep0_c[b])) &
                   (i_all[:, :, 0] < int(sep1_c[b])))
        cst[:, VMF + b * NT : VMF + (b + 1) * NT] = valid_i.astype(np.float32)
        cst[:, NM1 + b * NT : NM1 + (b + 1) * NT] = np.where(valid_i, 0.0, NEG)
        qn2 = (np.float32(np.dot(q1, q1)) + np.float32(np.dot(q2, q2))).astype(np.float32)
        rsq = np.float32(1.0) / np.sqrt(qn2, dtype=np.float32)
        cconst_l = (p[:, None] + 128 * np.arange(NT)[None, :] + L).astype(np.float32)
        cst[:, CCV + b * NT : CCV + (b + 1) * NT] = np.where(
            valid_i, cconst_l, -1.0)
        cst[:, VVR + b * NT : VVR + (b + 1) * NT] = np.where(
            valid_i, rsq, 0.0)
    cst[:, ID : ID + 128] = np.eye(128, dtype=np.float32)
    for j in range(128):
        cst[j, C1O + j] = 1.0
    for j in range(128, 160):
        cst[j - 128, C2O + j] = 1.0
    cst[:, RIO : RIO + NT * L] = np.broadcast_to(
        (L - np.arange(L))[None, None, :], (128, NT, L)).reshape(128, NT * L)
    cst[:, CC : CC + NT] = (p[:, None] + 128 * np.arange(NT)[None, :] + L)
    return dict(seq=np.ascontiguousarray(seq_c, dtype=np.float32), consts=cst)


def kernel(sequence_outputs, idxs, max_ans_len):
    seq = np.asarray(sequence_outputs, dtype=np.float32)
    idx = np.asarray(idxs).astype(np.int64)
    assert int(max_ans_len) == L and seq.shape == (32, S, H)

    if "nc" not in _cache:
        _cache["nc"] = _build()
    nc = _cache["nc"]

    in_maps = []
    for core in range(8):
        sl = slice(core * B_PER_CORE, (core + 1) * B_PER_CORE)
        in_maps.append(_prep_core(seq[sl], idx[sl, 0], idx[sl, 1]))

    res = run_bass_kernel_spmd(nc, in_maps, core_ids=list(range(8))).results
    mv = np.concatenate([r["mv_out"] for r in res], axis=0)
    ei = np.concatenate([r["ei_out"] for r in res], axis=0)
    return mv.astype(np.float32), ei.astype(np.int32)


# revision 4
# speedup vs baseline: 1.0643x; 1.0643x over previous
"""Trainium2 Bass kernel for the span-search problem (nn_DCR_21285857919673).

Data-parallel over batch: 32 batches / 8 cores = 4 per core. Per batch:
  n2[s]      square+accumulate over H (split ACT activation / DVE stt)
  d1,d2      PE: transpose 128x128 blocks -> PSUM -> SBUF, then reversed
             matmuls (lhsT = transposed block, rhs = [q1,q2]) giving
             d in [128(s), 2] layout directly (output free size 2).
  banded window (start i, len l<32) built ON-CHIP with PE shift-matmuls:
             win[p,r,c,l] = dn[p+l, r, c] with column wrap, via two
             identity-slice pieces per shift accumulated in one PSUM bank.
  mask via additive penalty constants; max / first-argmax / validity via
  DVE reductions + Pool elementwise; sqrt on ACT; Newton-refined divide.
The per-batch band phase is software-pipelined one batch behind the heavy
phase so long-latency waits never plug the in-order engine queues.
All sep-dependent values (q vectors, penalties, validity masks) are shipped
as one packed per-core constant tensor (same SPMD program on all 8 cores).
"""
import sys

sys.path.insert(0, "/opt/trn_rl_repo")

import numpy as np

import concourse.bass as bass
import concourse.bacc as bacc
import concourse.mybir as mybir
import concourse.tile as tile
from concourse.alu_op_type import AluOpType
from concourse import bass_isa
from concourse.bass_utils import run_bass_kernel_spmd

F32 = mybir.dt.float32
BF16 = mybir.dt.bfloat16
I32 = mybir.dt.int32

B_PER_CORE = 4
S = 1024
H = 1024
L = 32
NT = S // 128   # token tiles per batch
NC = H // 128   # h chunks
NEG = -10000.0
PENALTY = -16384.0   # additive mask penalty (exact in f32; way below any sim)

# consts column layout
QM = 0                    # qmat: (b*NC+c)*2+q            -> 64
ID = QM + B_PER_CORE * NC * 2        # ident 128
C1O = ID + 128                       # c1 160
C2O = C1O + 160                      # c2 160
PEN = C2O + 160                      # penj: 256*b + 32*c + l -> 1024
RIO = PEN + B_PER_CORE * NT * L      # riota 256
CC = RIO + NT * L                    # cconst 8
VMF = CC + NT                        # vmif 32
NM1 = VMF + B_PER_CORE * NT          # negm1 32
CCV = NM1 + B_PER_CORE * NT          # cconst*vmif + negm2 32
VVR = CCV + B_PER_CORE * NT          # rsq*vmif 32
CW = VVR + B_PER_CORE * NT           # total 1928

_cache = {}


def _bc(ap, n_mid, width):
    """[128, n_mid] AP -> [128, n_mid, width] broadcast along a new last dim."""
    return bass.AP(ap.tensor, ap.offset, [[ap.ap[0][0], 128], [1, n_mid], [0, width]])


def _build():
    nc = bacc.Bacc("TRN2", target_bir_lowering=False, debug=False)

    seq_in = nc.dram_tensor("seq", [B_PER_CORE, S, H], F32, kind="ExternalInput").ap()
    consts_in = nc.dram_tensor("consts", [128, CW], F32, kind="ExternalInput").ap()
    mv_out = nc.dram_tensor("mv_out", [B_PER_CORE, S], F32, kind="ExternalOutput").ap()
    ei_out = nc.dram_tensor("ei_out", [B_PER_CORE, S], I32, kind="ExternalOutput").ap()

    with tile.TileContext(nc) as tc:
        with (
            tc.tile_pool(name="inp", bufs=32) as inp,
            tc.tile_pool(name="cons", bufs=1) as cons,
            tc.tile_pool(name="trash", bufs=3) as trash_p,
            tc.tile_pool(name="stp", bufs=9) as stp,
            tc.tile_pool(name="band", bufs=3) as band_p,
            tc.tile_pool(name="small", bufs=3) as small_p,
            tc.tile_pool(name="outacc", bufs=1) as outacc,
            tc.tile_pool(name="tpp", bufs=4, space="PSUM") as tpp,
            tc.tile_pool(name="dpp", bufs=2, space="PSUM") as dpp,
            tc.tile_pool(name="wpp", bufs=2, space="PSUM") as wpp,
        ):
            cst = cons.tile([128, CW], F32, tag="cst")
            # split consts load: header (q/ident/shift mats) gates compute,
            # the rest (penalties etc) is only needed by the band phase
            nc.sync.dma_start(cst[:, 0:PEN], consts_in[:, 0:PEN])
            ident = cst[:, ID : ID + 128]
            c1 = cst[:, C1O : C1O + 160]
            c2 = cst[:, C2O : C2O + 160]
            riota3 = cst[:, RIO : RIO + NT * L].rearrange("p (c l) -> p c l", l=L)
            cconst = cst[:, CC : CC + NT]

            mvacc = outacc.tile([128, B_PER_CORE, NT], F32, tag="mvacc")
            eiacc = outacc.tile([128, B_PER_CORE, NT], I32, tag="eiacc")

            state = {}

            def emit_loads(b):
                tiles = []
                for t in range(NT):
                    tl = inp.tile([128, H], F32, tag="seq", name=f"tl{b}_{t}")
                    nc.sync.dma_start(tl[:], seq_in[b, bass.ts(t, 128), :])
                    tiles.append(tl[:])
                if b == 0:
                    nc.sync.dma_start(cst[:, PEN:CW], consts_in[:, PEN:CW])
                dn = small_p.tile([128, 2, NT + 1], F32, tag="dn", name=f"dn{b}")
                nc.gpsimd.memset(dn[:, :, NT : NT + 1], 0.0)
                for t in range(NT):
                    tr = trash_p.tile([128, H], BF16, tag="tr")
                    acc = dn[:, 1, t : t + 1]
                    if t % 2 == 0:
                        nc.scalar.activation(tr[:], tiles[t],
                                             mybir.ActivationFunctionType.Square,
                                             accum_out=acc)
                    else:
                        nc.vector.scalar_tensor_tensor(
                            out=tr[:], in0=tiles[t], scalar=1.0,
                            in1=tiles[t], op0=AluOpType.mult,
                            op1=AluOpType.mult, accum_out=acc)
                sts = [stp.tile([128, 1024], F32, tag="st", name=f"st{b}_{c}")
                       for c in range(NC)]
                dps = dpp.tile([128, NT, 2], F32, tag="dps", name=f"dps{b}")
                state[b] = dict(tiles=tiles, dn=dn, sts=sts, dps=dps)

            def emit_half(b, half):
                st_ = state[b]
                tiles, dn, sts, dps = st_["tiles"], st_["dn"], st_["sts"], st_["dps"]
                qmat_b = cst[:, QM + b * NC * 2 : QM + (b + 1) * NC * 2]
                for c in range(NC):
                    tp = tpp.tile([128, 512], F32, tag="tp")
                    for tt in range(4):
                        t = 4 * half + tt
                        nc.tensor.transpose(tp[:, bass.ts(tt, 128)],
                                            tiles[t][:, bass.ts(c, 128)], ident)
                    dst = sts[c][:, half * 512 : half * 512 + 512]
                    if c < 5:
                        nc.scalar.copy(dst, tp[:])
                    else:
                        nc.vector.tensor_copy(dst, tp[:])
                for t in range(4 * half, 4 * half + 4):
                    for c in range(NC):
                        nc.tensor.matmul(dps[:, t, :],
                                         lhsT=sts[c][:, bass.ts(t, 128)],
                                         rhs=qmat_b[:, 2 * c : 2 * c + 2],
                                         start=(c == 0), stop=(c == NC - 1))
                if half == 1:
                    d1col = small_p.tile([128, NT], F32, tag="d1col",
                                         name=f"d1c{b}")
                    nc.vector.tensor_copy(d1col[:], dps[:, :, 0])
                    nc.vector.tensor_copy(dn[:, 0, 0:NT], dps[:, :, 1])
                    st_["d1col"] = d1col

            def emit_win_n2row(b):
                st_ = state[b]
                dn = st_["dn"]
                win = wpp.tile([128, 2, NT, L], F32, tag="win", name=f"win{b}")
                # n2-row shifts first: they only need the n2 accumulations,
                # so den/rec can start while the d-row still waits on copies
                for l in range(L):
                    nc.tensor.matmul(win[:, 1, :, l], lhsT=c1[:, l : l + 128],
                                     rhs=dn[:, 1:2, 0:NT],
                                     start=(l == 0), stop=False)
                    nc.tensor.matmul(win[:, 1, :, l], lhsT=c2[:, l : l + 128],
                                     rhs=dn[:, 1:2, 1 : NT + 1],
                                     start=False, stop=False)
                n2ibc = bass.AP(dn.tensor, dn[:].offset + NT + 1,
                                [[dn[:].ap[0][0], 128], [1, NT], [0, L]])
                nc.tensor.matmul(win[:, 1], lhsT=ident, rhs=n2ibc,
                                 start=False, stop=False)
                st_["win"] = win

            def emit_win_drow(b):
                st_ = state[b]
                dn, win, d1col = st_["dn"], st_["win"], st_["d1col"]
                penj3 = cst[:, PEN + b * NT * L : PEN + (b + 1) * NT * L] \
                    .rearrange("p (c l) -> p c l", l=L)
                nc.tensor.matmul(win[:, 0], lhsT=ident, rhs=penj3,
                                 start=False, stop=False)
                nc.tensor.matmul(win[:, 0], lhsT=ident, rhs=_bc(d1col[:], NT, L),
                                 start=False, stop=False)
                for l in range(L):
                    nc.tensor.matmul(win[:, 0, :, l], lhsT=c1[:, l : l + 128],
                                     rhs=dn[:, 0:1, 0:NT],
                                     start=False, stop=False)
                    nc.tensor.matmul(win[:, 0, :, l], lhsT=c2[:, l : l + 128],
                                     rhs=dn[:, 0:1, 1 : NT + 1],
                                     start=False, stop=(l == L - 1))

            def emit_band1(b):
                st_ = state[b]
                win = st_["win"]
                den = band_p.tile([128, NT, L], F32, tag="den")
                nc.scalar.sqrt(den[:], win[:, 1])
                rec = band_p.tile([128, NT, L], F32, tag="rec")
                nc.vector.reciprocal(rec[:], den[:])
                st_.update(den=den, rec=rec)

            def emit_band2(b):
                st_ = state[b]
                numer, den, rec, rsq = st_["numer"], st_["den"], st_["rec"], st_["rsq"]
                vmif_b = cst[:, VMF + b * NT : VMF + (b + 1) * NT]
                negm1_b = cst[:, NM1 + b * NT : NM1 + (b + 1) * NT]
                ccvn_b = cst[:, CCV + b * NT : CCV + (b + 1) * NT]
                vvr_b = cst[:, VVR + b * NT : VVR + (b + 1) * NT]
                sim0 = band_p.tile([128, NT, L], F32, tag="sim0")
                nc.gpsimd.tensor_tensor(out=sim0[:], in0=numer[:], in1=rec[:],
                                        op=AluOpType.mult)
                nt1 = band_p.tile([128, NT, L], F32, tag="nt1")
                nc.gpsimd.tensor_tensor(out=nt1[:], in0=sim0[:], in1=den[:],
                                        op=AluOpType.mult)
                nt2 = band_p.tile([128, NT, L], F32, tag="nt2")
                nc.gpsimd.tensor_tensor(out=nt2[:], in0=numer[:], in1=nt1[:],
                                        op=AluOpType.subtract)
                nt3 = band_p.tile([128, NT, L], F32, tag="nt3")
                nc.gpsimd.tensor_tensor(out=nt3[:], in0=nt2[:], in1=rec[:],
                                        op=AluOpType.mult)
                sim = band_p.tile([128, NT, L], F32, tag="sim")
                nc.gpsimd.tensor_tensor(out=sim[:], in0=sim0[:], in1=nt3[:],
                                        op=AluOpType.add)
                maxv = small_p.tile([128, NT], F32, tag="maxv")
                nc.vector.tensor_reduce(out=maxv[:], in_=sim[:],
                                        axis=mybir.AxisListType.X, op=AluOpType.max)
                eq = band_p.tile([128, NT, L], F32, tag="eq")
                nc.vector.tensor_tensor(out=eq[:], in0=sim[:],
                                        in1=_bc(maxv[:], NT, L), op=AluOpType.is_equal)
                wt = band_p.tile([128, NT, L], F32, tag="wt")
                nc.gpsimd.tensor_tensor(out=wt[:], in0=eq[:], in1=riota3,
                                        op=AluOpType.mult)
                mval = small_p.tile([128, NT], F32, tag="mval")
                nc.vector.tensor_reduce(out=mval[:], in_=wt[:],
                                        axis=mybir.AxisListType.X, op=AluOpType.max)
                mvt = small_p.tile([128, NT], F32, tag="mvt")
                eng.tensor_tensor(out=mvt[:], in0=maxv[:], in1=vvr_b,
                                  op=AluOpType.mult)
                eng.tensor_tensor(out=mvacc[:, b, :], in0=mvt[:], in1=negm1_b,
                                  op=AluOpType.add)
                vm1 = small_p.tile([128, NT], F32, tag="vm1")
                eng.tensor_tensor(out=vm1[:], in0=mval[:], in1=vmif_b,
                                  op=AluOpType.mult)
                ef2 = small_p.tile([128, NT], F32, tag="ef2")
                eng.tensor_tensor(out=ef2[:], in0=ccvn_b, in1=vm1[:],
                                  op=AluOpType.subtract)
                nc.gpsimd.tensor_copy(eiacc[:, b, :], ef2[:])
                del state[b]

            # fine-grained software pipeline
            for b in range(B_PER_CORE):
                emit_loads(b)
                if b > 0:
                    emit_band1(b - 1)
                emit_qn2(b)
                emit_half(b, 0)
                if b > 0:
                    emit_band2(b - 1)
                emit_half(b, 1)
                emit_win(b)
            emit_band1(B_PER_CORE - 1)
            emit_band2(B_PER_CORE - 1)

    nc.compile()
    return nc


def _prep_core(seq_c, sep0_c, sep1_c):
    """Host-side constant prep for one core. seq_c: [4, S, H] f32."""
    Bc = seq_c.shape[0]
    p = np.arange(128)
    cst = np.zeros((128, CW), np.float32)
    for b in range(Bc):
        q1 = seq_c[b, 1, :]
        q2 = seq_c[b, int(sep0_c[b]) - 1, :]
        for c in range(NC):
            cst[:, QM + (b * NC + c) * 2] = q1[c * 128 : (c + 1) * 128]
            cst[:, QM + (b * NC + c) * 2 + 1] = q2[c * 128 : (c + 1) * 128]
        i_all = p[:, None, None] + 128 * np.arange(NT)[None, :, None]  # [128, NT, 1]
        j_all = i_all + np.arange(L)[None, None, :]                    # [128, NT, L]
        pen = np.where(j_all < int(sep1_c[b]), 0.0, PENALTY).astype(np.float32)
        cst[:, PEN + b * NT * L : PEN + (b + 1) * NT * L] = pen.reshape(128, NT * L)
        valid_i = ((i_all[:, :, 0] > int(sep0_c[b])) &
                   (i_all[:, :, 0] < int(sep1_c[b])))
        cst[:, VMF + b * NT : VMF + (b + 1) * NT] = valid_i.astype(np.float32)
        cst[:, NM1 + b * NT : NM1 + (b + 1) * NT] = np.where(valid_i, 0.0, NEG)
        qn2 = (np.float32(np.dot(q1, q1)) + np.float32(np.dot(q2, q2))).astype(np.float32)
        rsq = np.float32(1.0) / np.sqrt(qn2, dtype=np.float32)
        cconst_l = (p[:, None] + 128 * np.arange(NT)[None, :] + L).astype(np.float32)
        cst[:, CCV + b * NT : CCV + (b + 1) * NT] = np.where(
            valid_i, cconst_l, -1.0)
        cst[:, VVR + b * NT : VVR + (b + 1) * NT] = np.where(
            valid_i, rsq, 0.0)
    cst[:, ID : ID + 128] = np.eye(128, dtype=np.float32)
    for j in range(128):
        cst[j, C1O + j] = 1.0
    for j in range(128, 160):
        cst[j - 128, C2O + j] = 1.0
    cst[:, RIO : RIO + NT * L] = np.broadcast_to(
        (L - np.arange(L))[None, None, :], (128, NT, L)).reshape(128, NT * L)
    cst[:, CC : CC + NT] = (p[:, None] + 128 * np.arange(NT)[None, :] + L)
    return dict(seq=np.ascontiguousarray(seq_c, dtype=np.float32), consts=cst)


def kernel(sequence_outputs, idxs, max_ans_len):
    seq = np.asarray(sequence_outputs, dtype=np.float32)
    idx = np.asarray(idxs).astype(np.int64)
    assert int(max_ans_len) == L and seq.shape == (32, S, H)

    if "nc" not in _cache:
        _cache["nc"] = _build()
    nc = _cache["nc"]

    in_maps = []
    for core in range(8):
        sl = slice(core * B_PER_CORE, (core + 1) * B_PER_CORE)
        in_maps.append(_prep_core(seq[sl], idx[sl, 0], idx[sl, 1]))

    res = run_bass_kernel_spmd(nc, in_maps, core_ids=list(range(8))).results
    mv = np.concatenate([r["mv_out"] for r in res], axis=0)
    ei = np.concatenate([r["ei_out"] for r in res], axis=0)
    return mv.astype(np.float32), ei.astype(np.int32)
